# revision 4
# baseline (speedup 1.0000x reference)
"""Trainium2 Bass kernel for nn_Attention_48610439856262.

Gated attention block:
    qkv = x @ W_qkv ; gate = x @ W_gate ; s = e @ W_s (added to k)
    attn = softmax(q @ (k+s).T * D**-0.5) ; out = (attn @ v) * gate
    y = out @ W_proj + b_proj

Sharding (8 cores, tensor-parallel over heads):
  Core c owns heads {2c, 2c+1} = feature columns 128c:128c+128 of the
  (H, D)-structured feature axis.  Each core computes q/k+s/v/gate for its
  128 feature columns over all 4096 tokens, runs attention for its 2 heads,
  multiplies by its gate slice, and computes a PARTIAL projection
  y_c = gated_c @ W_proj[128c:128c+128, :]  ->  [4096, 1024].
  The host sums the 8 partials and adds b_proj (no device collectives).

Layout/precision notes (all activations bf16 on the wire and in SBUF;
PSUM accumulation fp32):
  xT, eT   [1024, 4096] bf16 (host pre-transposes + downcasts)
  qT/kpsT/gT/gatedT  SBUF [128 feat, 4096 tok] bf16; k+s is fused into a
           single PSUM accumulation group (16 matmuls), no DVE add.
  v        computed token-major directly (x-chunk as stationary, W_v as
           moving operand) -> no PE transposes; stored per 128-token block
           as [v_h0(64) | ones | v_h1(64) | ones] so the attn @ v_aug PSUM
           row 64 holds the softmax denominators, partition-aligned with
           the gate slice for h0 (h1 uses one SBUF->SBUF DMA shift).
  scores   PSUM [128 keys, 1024] fp32 = [h0 512q | h1 512q]: the two
           heads' score matmuls are row-tiled (contraction 64: partitions
           0-63 vs 64-127 -> tile_position (0,0)/(64,0)) and adjacent in
           issue order, so they overlap on the PE array; one 1024-wide
           ACT Exp call (fused *SCALE, no max-subtraction: scores are
           ~N(0, 0.6), |s|<6, exp is safe in fp32) covers both heads.
  Phase order is batch-pipelined: A(b0) -> B(b0) || A(b1) -> B(b1), with
  the projection interleaved per (b, nh) reusing the pv PSUM banks.
  PSUM budget: scores 2x[128,1024] (4 banks) + pv0/pv1 [128,512] (2) +
  acc (1) + vacc (1) = 8.
"""

import numpy as np
import ml_dtypes

BF16 = ml_dtypes.bfloat16

B, N, C, H, D = 2, 2048, 1024, 16, 64
T = B * N              # 4096 tokens
NCORES = 8
F = 128                # feature columns per core (2 heads x 64)
SCALE = D ** -0.5
KC = C // 128          # 8 contraction chunks
TC = T // 512          # 8 token chunks of 512
MB = N // 128          # 16 key blocks per sequence
TB = T // 128          # 32 token blocks

_cache: dict = {}


def _build_program(reps=1):
    import concourse.bacc as bacc
    import concourse.tile as tile
    from concourse import mybir

    f32 = mybir.dt.float32
    bf16 = mybir.dt.bfloat16

    nc = bacc.Bacc("TRN2", target_bir_lowering=False, debug=False,
                   num_devices=NCORES)

    xT = nc.dram_tensor("xT", [C, T], bf16, kind="ExternalInput").ap()
    eT = nc.dram_tensor("eT", [C, T], bf16, kind="ExternalInput").ap()
    wq = nc.dram_tensor("wq", [C, F], bf16, kind="ExternalInput").ap()
    wk = nc.dram_tensor("wk", [C, F], bf16, kind="ExternalInput").ap()
    wv = nc.dram_tensor("wv", [C, F], bf16, kind="ExternalInput").ap()
    ws = nc.dram_tensor("ws", [C, F], bf16, kind="ExternalInput").ap()
    wg = nc.dram_tensor("wg", [C, F], bf16, kind="ExternalInput").ap()
    wp = nc.dram_tensor("wp", [F, C], bf16, kind="ExternalInput").ap()
    y = nc.dram_tensor("y", [T, C], bf16, kind="ExternalOutput").ap()

    Exp = mybir.ActivationFunctionType.Exp

    with tile.TileContext(nc) as tc:
        with tc.tile_pool(name="persist", bufs=1) as persist, \
             tc.tile_pool(name="psum", bufs=1, space="PSUM") as psum, \
             tc.tile_pool(name="xa", bufs=10) as xa_pool, \
             tc.tile_pool(name="ea", bufs=10) as ea_pool, \
             tc.tile_pool(name="pt", bufs=6) as pt_pool, \
             tc.tile_pool(name="small", bufs=3) as small, \
             tc.tile_pool(name="yout", bufs=4) as y_pool:
            # Weights, contraction-chunked: [128 k-part, KC, 128 cols]
            w_sb = {}
            for name, src in (("wq", wq), ("wk", wk), ("wv", wv),
                              ("ws", ws), ("wg", wg)):
                t_ = persist.tile([128, KC, F], bf16, tag=name, name=f"w_{name}")
                nc.sync.dma_start(out=t_,
                                  in_=src.rearrange("(k p) f -> p k f", p=128))
                w_sb[name] = t_
            wp_sb = persist.tile([F, C], bf16, tag="wp")
            nc.sync.dma_start(out=wp_sb, in_=wp)

            qT_s = persist.tile([128, T], bf16, tag="qT")
            kpsT_s = persist.tile([128, T], bf16, tag="kpsT")
            gT_s = persist.tile([128, T], bf16, tag="gT")
            gatedT_s = persist.tile([128, T], bf16, tag="gatedT")
            # v_aug per 128-token block: [v_h0 | 1 | v_h1 | 1] -> the
            # attn@v_aug PSUM row 64 is the softmax denominator.
            v_s = persist.tile([128, TB, 130], bf16, tag="v")
            ones_col = persist.tile([128, TB], bf16, tag="ones_col")
            nc.vector.memset(ones_col, 1.0)
            nc.vector.tensor_copy(v_s[:, :, 64], ones_col)
            nc.vector.tensor_copy(v_s[:, :, 129], ones_col)

            for _rep in range(reps):
                for b in range(B):
                    # ---- Phase A(b): projections for batch b's tokens ----
                    for t in range(b * TC // B, (b + 1) * TC // B):
                        sl = slice(t * 512, (t + 1) * 512)
                        xt = [xa_pool.tile([128, 512], bf16, tag="xt",
                                           name=f"xt{t}_{k}")
                              for k in range(KC)]
                        for k in range(KC):
                            nc.sync.dma_start(
                                out=xt[k], in_=xT[k * 128:(k + 1) * 128, sl])
                        et = [ea_pool.tile([128, 512], bf16, tag="et",
                                           name=f"et{t}_{k}")
                              for k in range(KC)]
                        for k in range(KC):
                            nc.sync.dma_start(
                                out=et[k], in_=eT[k * 128:(k + 1) * 128, sl])
                        # q, gate: feature-major [128 feat, 512 tok]
                        for out_name, dst in (("q", qT_s), ("g", gT_s)):
                            acc = psum.tile([128, 512], f32, tag="acc",
                                            name=f"acc_{out_name}")
                            w_t = w_sb["w" + out_name]
                            for k in range(KC):
                                nc.tensor.matmul(acc, w_t[:, k, :], xt[k],
                                                 start=(k == 0),
                                                 stop=(k == KC - 1))
                            nc.vector.tensor_copy(dst[:, sl], acc)
                        # k+s fused in one PSUM accumulation group
                        acc = psum.tile([128, 512], f32, tag="acc",
                                        name="acc_kps")
                        for k in range(KC):
                            nc.tensor.matmul(acc, w_sb["wk"][:, k, :], xt[k],
                                             start=(k == 0), stop=False)
                        for k in range(KC):
                            nc.tensor.matmul(acc, w_sb["ws"][:, k, :], et[k],
                                             start=False, stop=(k == KC - 1))
                        nc.vector.tensor_copy(kpsT_s[:, sl], acc)
                        # v token-major: x-chunk stationary, W_v moving
                        vacc = psum.tile([128, 4, 128], f32, tag="vacc",
                                         name="vacc")
                        for j in range(4):
                            for k in range(KC):
                                nc.tensor.matmul(
                                    vacc[:, j, :],
                                    xt[k][:, j * 128:(j + 1) * 128],
                                    w_sb["wv"][:, k, :],
                                    start=(k == 0), stop=(k == KC - 1))
                        for j in range(4):
                            tb = t * 4 + j
                            nc.vector.tensor_copy(v_s[:, tb, 0:64],
                                                  vacc[:, j, 0:64])
                            nc.vector.tensor_copy(v_s[:, tb, 65:129],
                                                  vacc[:, j, 64:128])

                    # ---- Phase B(b): attention + gating + projection ----
                    for nh in range(2):
                        for jj in range(2):
                            nsl = slice(b * N + nh * 1024 + jj * 512,
                                        b * N + nh * 1024 + (jj + 1) * 512)
                            psv = [psum.tile([128, 512], f32, tag=f"pv{h}",
                                             name=f"pv{h}") for h in range(2)]
                            # software-pipelined: emit scores(mb+1) before
                            # PV(mb) so the PE never stalls at a PV that
                            # waits on ACT(mb) while scores(mb+1) is ready.
                            pts = {}

                            def emit_scores(mb, b=b, nsl=nsl, pts=pts):
                                msl = slice(b * N + mb * 128,
                                            b * N + mb * 128 + 128)
                                ps_s = psum.tile([128, 1024], f32,
                                                 tag="scores", name="scores",
                                                 bufs=2)
                                # two heads row-tiled: contraction rows
                                # 0-63 / 64-127, adjacent in issue order
                                for h in range(2):
                                    hsl = slice(h * 64, (h + 1) * 64)
                                    nc.tensor.matmul(
                                        ps_s[:, h * 512:(h + 1) * 512],
                                        kpsT_s[hsl, msl], qT_s[hsl, nsl],
                                        start=True, stop=True)
                                pt = pt_pool.tile([128, 1024], bf16, tag="pT")
                                nc.scalar.activation(pt, ps_s, Exp,
                                                     scale=SCALE)
                                pts[mb] = pt

                            def emit_pv(mb, b=b, psv=psv, pts=pts):
                                pt = pts.pop(mb)
                                for h in range(2):
                                    nc.tensor.matmul(
                                        psv[h][0:65, :],
                                        v_s[:, b * MB + mb,
                                            h * 65:h * 65 + 65],
                                        pt[:, h * 512:(h + 1) * 512],
                                        start=(mb == 0), stop=(mb == MB - 1))

                            emit_scores(0)
                            for mb in range(MB):
                                if mb + 1 < MB:
                                    emit_scores(mb + 1)
                                emit_pv(mb)
                            for h in range(2):
                                hsl = slice(h * 64, (h + 1) * 64)
                                rs = small.tile([1, 512], f32, tag="rs")
                                nc.vector.reciprocal(rs, psv[h][64:65, :])
                                rb = small.tile([64, 512], f32, tag="rb")
                                nc.gpsimd.partition_broadcast(rb, rs)
                                if h == 0:
                                    # partitions already aligned with gate
                                    tmp = small.tile([64, 512], f32,
                                                     tag="tmp")
                                    nc.vector.tensor_mul(tmp, psv[0][0:64, :],
                                                         rb)
                                    nc.vector.tensor_mul(
                                        gatedT_s[hsl, nsl], tmp,
                                        gT_s[hsl, nsl])
                                else:
                                    tmp = small.tile([64, 512], f32,
                                                     tag="tmp")
                                    nc.vector.tensor_mul(tmp, psv[1][0:64, :],
                                                         rb)
                                    pvn = small.tile([128, 512], f32,
                                                     tag="pvn")
                                    nc.sync.dma_start(out=pvn[hsl, :],
                                                      in_=tmp)
                                    nc.vector.tensor_mul(
                                        gatedT_s[hsl, nsl], pvn[hsl, :],
                                        gT_s[hsl, nsl])
                        # projection for this (b, nh): reuses pv banks
                        for tb in range(b * 16 + nh * 8, b * 16 + nh * 8 + 8):
                            for j in range(2):
                                py_ = psum.tile([128, 512], f32,
                                                tag=f"pv{j}", name="proj")
                                nc.tensor.matmul(
                                    py_,
                                    gatedT_s[:, tb * 128:(tb + 1) * 128],
                                    wp_sb[:, j * 512:(j + 1) * 512],
                                    start=True, stop=True)
                                yt = y_pool.tile([128, 512], bf16, tag="yt")
                                nc.vector.tensor_copy(yt, py_)
                                nc.sync.dma_start(
                                    out=y[tb * 128:(tb + 1) * 128,
                                          j * 512:(j + 1) * 512],
                                    in_=yt)

    nc.compile()
    return nc


def _get_nc():
    if "nc" not in _cache:
        _cache["nc"] = _build_program()
    return _cache["nc"]


def _get_exec():
    """Compile once; cache a persistent sharded executable.

    Mirrors concourse.bass2jax.run_bass_via_pjrt's multi-core path, but
    keeps the jitted callable (and device-resident zero output buffers)
    alive so repeat kernel() calls skip XLA/walrus recompilation.  No
    donation: the kernel writes every element of y, so the zero buffers
    are never read and can be reused across calls.
    """
    if "exec" in _cache:
        return _cache["exec"]
    import jax
    from jax.experimental.shard_map import shard_map
    from jax.sharding import Mesh, PartitionSpec
    from concourse import mybir
    from concourse.bass2jax import (_bass_exec_p, install_neuronx_cc_hook,
                                    partition_id_tensor)

    nc = _get_nc()
    install_neuronx_cc_hook()
    partition_name = (nc.partition_id_tensor.name
                      if nc.partition_id_tensor else None)
    in_names, out_names, out_avals = [], [], []
    for alloc in nc.m.functions[0].allocations:
        if not isinstance(alloc, mybir.MemoryLocationSet):
            continue
        name = alloc.memorylocations[0].name
        if alloc.kind == "ExternalInput":
            if name != partition_name:
                in_names.append(name)
        elif alloc.kind == "ExternalOutput":
            out_names.append(name)
            out_avals.append(jax.core.ShapedArray(
                tuple(alloc.tensor_shape), mybir.dt.np(alloc.dtype)))
    n_params, n_outs = len(in_names), len(out_names)
    bind_in_names = tuple(in_names + out_names +
                          ([partition_name] if partition_name else []))

    def _body(*args):
        operands = list(args)
        if partition_name is not None:
            operands.append(partition_id_tensor())
        outs = _bass_exec_p.bind(
            *operands,
            out_avals=tuple(out_avals),
            in_names=bind_in_names,
            out_names=tuple(out_names),
            lowering_input_output_aliases=(),
            sim_require_finite=True,
            sim_require_nnan=True,
            nc=nc,
        )
        return tuple(outs)

    devices = jax.devices()[:NCORES]
    mesh = Mesh(np.asarray(devices), ("core",))
    in_specs = (PartitionSpec("core"),) * (n_params + n_outs)
    out_specs = (PartitionSpec("core"),) * n_outs
    sharded = jax.jit(shard_map(_body, mesh=mesh, in_specs=in_specs,
                                out_specs=out_specs, check_rep=False),
                      keep_unused=True)
    zeros_dev = [
        jax.device_put(
            np.zeros((NCORES * a.shape[0], *a.shape[1:]), a.dtype),
            jax.sharding.NamedSharding(mesh, PartitionSpec("core")))
        for a in out_avals]
    reduce_fn = jax.jit(
        lambda a: a.reshape(NCORES, T, C).astype(np.float32).sum(axis=0))
    ex = {"fn": sharded, "in_names": in_names, "out_names": out_names,
          "out_avals": out_avals, "mesh": mesh, "zeros_dev": zeros_dev,
          "spec": PartitionSpec("core"), "reduce": reduce_fn}
    _cache["exec"] = ex
    return ex


def _run_on_device(in_maps):
    """Run the cached executable; returns per-core output dicts."""
    ex = _get_exec()
    concat_in = [
        np.concatenate([np.asarray(in_maps[c][name])
                        for c in range(NCORES)], axis=0)
        for name in ex["in_names"]]
    out = ex["fn"](*concat_in, *ex["zeros_dev"])
    return [
        {name: np.asarray(out[i]).reshape(NCORES, *ex["out_avals"][i].shape)[c]
         for i, name in enumerate(ex["out_names"])}
        for c in range(NCORES)]


def _make_in_maps(x, e, W_qkv, W_s, W_gate, W_proj):
    xT = np.ascontiguousarray(
        x.reshape(T, C).T).astype(BF16)
    eT = np.ascontiguousarray(
        e.reshape(T, C).T).astype(BF16)
    in_maps = []
    for c in range(NCORES):
        fs = slice(F * c, F * (c + 1))
        in_maps.append({
            "xT": xT,
            "eT": eT,
            "wq": np.ascontiguousarray(W_qkv[:, fs]).astype(BF16),
            "wk": np.ascontiguousarray(W_qkv[:, C:][:, fs]).astype(BF16),
            "wv": np.ascontiguousarray(W_qkv[:, 2 * C:][:, fs]).astype(BF16),
            "ws": np.ascontiguousarray(W_s[:, fs]).astype(BF16),
            "wg": np.ascontiguousarray(W_gate[:, fs]).astype(BF16),
            "wp": np.ascontiguousarray(W_proj[fs, :]).astype(BF16),
        })
    return in_maps


def kernel(x, e, W_qkv, W_s, W_gate, W_proj, b_proj):
    ex = _get_exec()
    in_maps = _make_in_maps(np.asarray(x), np.asarray(e), np.asarray(W_qkv),
                            np.asarray(W_s), np.asarray(W_gate),
                            np.asarray(W_proj))
    concat_in = [
        np.concatenate([np.asarray(in_maps[c][name])
                        for c in range(NCORES)], axis=0)
        for name in ex["in_names"]]
    out = ex["fn"](*concat_in, *ex["zeros_dev"])
    iy = ex["out_names"].index("y")
    y_sum = np.asarray(ex["reduce"](out[iy]))   # cross-core partial sum
    y_sum = y_sum + np.asarray(b_proj, dtype=np.float32)
    return y_sum.reshape(B, N, C).astype(np.float32)


# revision 8
# speedup vs baseline: 1.5639x; 1.5639x over previous
"""Trainium2 Bass kernel for nn_Attention_48610439856262.

Gated attention block:
    qkv = x @ W_qkv ; gate = x @ W_gate ; s = e @ W_s (added to k)
    attn = softmax(q @ (k+s).T * D**-0.5) ; out = (attn @ v) * gate
    y = out @ W_proj + b_proj

Sharding (8 cores, tensor-parallel over heads):
  Core c owns heads {2c, 2c+1} = feature columns 128c:128c+128 of the
  (H, D)-structured feature axis.  Each core computes q/k+s/v/gate for its
  128 feature columns over all 4096 tokens, runs attention for its 2 heads,
  multiplies by its gate slice, and computes a PARTIAL projection
  y_c = gated_c @ W_proj[128c:128c+128, :]  ->  [4096, 1024].
  The host sums the 8 partials and adds b_proj (no device collectives).

Layout/precision notes (all activations bf16 on the wire and in SBUF;
PSUM accumulation fp32):
  xT, eT   [1024, 4096] bf16 (host pre-transposes + downcasts)
  qT/kpsT/gT/gatedT  SBUF [128 feat, 4096 tok] bf16; k+s is fused into a
           single PSUM accumulation group (16 matmuls), no DVE add.
  v        computed token-major directly (x-chunk as stationary, W_v as
           moving operand) -> no PE transposes; stored per 128-token block
           as [v_h0(64) | ones | v_h1(64) | ones] so the attn @ v_aug PSUM
           row 64 holds the softmax denominators, partition-aligned with
           the gate slice for h0 (h1 uses one SBUF->SBUF DMA shift).
  scores   PSUM [128 keys, 1024] fp32 = [h0 512q | h1 512q]: the two
           heads' score matmuls are row-tiled (contraction 64: partitions
           0-63 vs 64-127 -> tile_position (0,0)/(64,0)) and adjacent in
           issue order, so they overlap on the PE array; one 1024-wide
           ACT Exp call (fused *SCALE, no max-subtraction: scores are
           ~N(0, 0.6), |s|<6, exp is safe in fp32) covers both heads.
  Phase order is batch-pipelined: A(b0) -> B(b0) || A(b1) -> B(b1), with
  the projection interleaved per (b, nh) reusing the pv PSUM banks.
  PSUM budget: scores 2x[128,1024] (4 banks) + pv0/pv1 [128,512] (2) +
  acc (1) + vacc (1) = 8.
"""

import os
import numpy as np
import ml_dtypes

USE_BF16 = os.environ.get("BASS_DT", "bf16") != "f32r"
BF16 = ml_dtypes.bfloat16 if USE_BF16 else np.float32

B, N, C, H, D = 2, 2048, 1024, 16, 64
T = B * N              # 4096 tokens
NCORES = 8
F = 128                # feature columns per core (2 heads x 64)
SCALE = D ** -0.5
KC = C // 128          # 8 contraction chunks
TC = T // 512          # 8 token chunks of 512
MB = N // 128          # 16 key blocks per sequence
TB = T // 128          # 32 token blocks

_cache: dict = {}


def _build_program(reps=1):
    import concourse.bacc as bacc
    import concourse.tile as tile
    from concourse import mybir

    f32 = mybir.dt.float32
    bf16 = mybir.dt.bfloat16 if USE_BF16 else mybir.dt.float32r

    nc = bacc.Bacc("TRN2", target_bir_lowering=False, debug=False,
                   num_devices=NCORES)

    xT = nc.dram_tensor("xT", [C, T], bf16, kind="ExternalInput").ap()
    eT = nc.dram_tensor("eT", [C, T], bf16, kind="ExternalInput").ap()
    wq = nc.dram_tensor("wq", [C, F], bf16, kind="ExternalInput").ap()
    wk = nc.dram_tensor("wk", [C, F], bf16, kind="ExternalInput").ap()
    wv = nc.dram_tensor("wv", [C, F], bf16, kind="ExternalInput").ap()
    ws = nc.dram_tensor("ws", [C, F], bf16, kind="ExternalInput").ap()
    wg = nc.dram_tensor("wg", [C, F], bf16, kind="ExternalInput").ap()
    wp = nc.dram_tensor("wp", [F, C], bf16, kind="ExternalInput").ap()
    y_dt = bf16 if USE_BF16 else f32
    y = nc.dram_tensor("y", [T, C], y_dt, kind="ExternalOutput").ap()

    Exp = mybir.ActivationFunctionType.Exp

    with tile.TileContext(nc) as tc:
        with tc.tile_pool(name="persist", bufs=1) as persist, \
             tc.tile_pool(name="psum", bufs=1, space="PSUM") as psum, \
             tc.tile_pool(name="xa", bufs=10) as xa_pool, \
             tc.tile_pool(name="ea", bufs=10) as ea_pool, \
             tc.tile_pool(name="pt", bufs=6) as pt_pool, \
             tc.tile_pool(name="small", bufs=3) as small, \
             tc.tile_pool(name="yout", bufs=4) as y_pool:
            # Weights, contraction-chunked: [128 k-part, KC, 128 cols]
            w_sb = {}
            for name, src in (("wq", wq), ("wk", wk), ("wv", wv),
                              ("ws", ws), ("wg", wg)):
                t_ = persist.tile([128, KC, F], bf16, tag=name, name=f"w_{name}")
                nc.sync.dma_start(out=t_,
                                  in_=src.rearrange("(k p) f -> p k f", p=128))
                w_sb[name] = t_
            wp_sb = persist.tile([F, C], bf16, tag="wp")
            nc.sync.dma_start(out=wp_sb, in_=wp)

            qT_s = persist.tile([128, T], bf16, tag="qT")
            kpsT_s = persist.tile([128, T], bf16, tag="kpsT")
            gT_s = persist.tile([128, T], bf16, tag="gT")
            gatedT_s = persist.tile([128, T], bf16, tag="gatedT")
            # v_aug per 128-token block: [v_h0 | 1 | v_h1 | 1] -> the
            # attn@v_aug PSUM row 64 is the softmax denominator.
            v_s = persist.tile([128, TB, 130], bf16, tag="v")
            ones_col = persist.tile([128, TB], bf16 if USE_BF16 else f32,
                                    tag="ones_col")
            nc.vector.memset(ones_col, 1.0)
            nc.vector.tensor_copy(v_s[:, :, 64], ones_col)
            nc.vector.tensor_copy(v_s[:, :, 129], ones_col)

            for _rep in range(reps):
                for b in range(B):
                    # ---- Phase A(b): projections for batch b's tokens ----
                    for t in range(b * TC // B, (b + 1) * TC // B):
                        sl = slice(t * 512, (t + 1) * 512)
                        xt = [xa_pool.tile([128, 512], bf16, tag="xt",
                                           name=f"xt{t}_{k}")
                              for k in range(KC)]
                        for k in range(KC):
                            nc.sync.dma_start(
                                out=xt[k], in_=xT[k * 128:(k + 1) * 128, sl])
                        et = [ea_pool.tile([128, 512], bf16, tag="et",
                                           name=f"et{t}_{k}")
                              for k in range(KC)]
                        for k in range(KC):
                            nc.sync.dma_start(
                                out=et[k], in_=eT[k * 128:(k + 1) * 128, sl])
                        # q, gate: feature-major [128 feat, 512 tok]
                        for out_name, dst in (("q", qT_s), ("g", gT_s)):
                            acc = psum.tile([128, 512], f32, tag="acc",
                                            name=f"acc_{out_name}")
                            w_t = w_sb["w" + out_name]
                            for k in range(KC):
                                nc.tensor.matmul(acc, w_t[:, k, :], xt[k],
                                                 start=(k == 0),
                                                 stop=(k == KC - 1))
                            nc.vector.tensor_copy(dst[:, sl], acc)
                        # k+s fused in one PSUM accumulation group
                        acc = psum.tile([128, 512], f32, tag="acc",
                                        name="acc_kps")
                        for k in range(KC):
                            nc.tensor.matmul(acc, w_sb["wk"][:, k, :], xt[k],
                                             start=(k == 0), stop=False)
                        for k in range(KC):
                            nc.tensor.matmul(acc, w_sb["ws"][:, k, :], et[k],
                                             start=False, stop=(k == KC - 1))
                        nc.vector.tensor_copy(kpsT_s[:, sl], acc)
                        # v token-major: x-chunk stationary, W_v moving
                        vacc = psum.tile([128, 4, 128], f32, tag="vacc",
                                         name="vacc")
                        for j in range(4):
                            for k in range(KC):
                                nc.tensor.matmul(
                                    vacc[:, j, :],
                                    xt[k][:, j * 128:(j + 1) * 128],
                                    w_sb["wv"][:, k, :],
                                    start=(k == 0), stop=(k == KC - 1))
                        for j in range(4):
                            tb = t * 4 + j
                            nc.vector.tensor_copy(v_s[:, tb, 0:64],
                                                  vacc[:, j, 0:64])
                            nc.vector.tensor_copy(v_s[:, tb, 65:129],
                                                  vacc[:, j, 64:128])

                    # ---- Phase B(b): attention + gating + projection ----
                    for nh in range(2):
                        for jj in range(2):
                            nsl = slice(b * N + nh * 1024 + jj * 512,
                                        b * N + nh * 1024 + (jj + 1) * 512)
                            psv = [psum.tile([128, 512], f32, tag=f"pv{h}",
                                             name=f"pv{h}") for h in range(2)]
                            # software-pipelined: emit scores(mb+1) before
                            # PV(mb) so the PE never stalls at a PV that
                            # waits on ACT(mb) while scores(mb+1) is ready.
                            pts = {}

                            def emit_scores(mb, b=b, nsl=nsl, pts=pts):
                                msl = slice(b * N + mb * 128,
                                            b * N + mb * 128 + 128)
                                ps_s = psum.tile([128, 1024], f32,
                                                 tag="scores", name="scores",
                                                 bufs=2)
                                # two heads row-tiled: contraction rows
                                # 0-63 / 64-127, adjacent in issue order
                                for h in range(2):
                                    hsl = slice(h * 64, (h + 1) * 64)
                                    nc.tensor.matmul(
                                        ps_s[:, h * 512:(h + 1) * 512],
                                        kpsT_s[hsl, msl], qT_s[hsl, nsl],
                                        start=True, stop=True)
                                pt = pt_pool.tile([128, 1024], bf16, tag="pT")
                                nc.scalar.activation(pt, ps_s, Exp,
                                                     scale=SCALE)
                                pts[mb] = pt

                            def emit_pv(mb, b=b, psv=psv, pts=pts):
                                pt = pts.pop(mb)
                                for h in range(2):
                                    nc.tensor.matmul(
                                        psv[h][0:65, :],
                                        v_s[:, b * MB + mb,
                                            h * 65:h * 65 + 65],
                                        pt[:, h * 512:(h + 1) * 512],
                                        start=(mb == 0), stop=(mb == MB - 1))

                            emit_scores(0)
                            for mb in range(MB):
                                if mb + 1 < MB:
                                    emit_scores(mb + 1)
                                emit_pv(mb)
                            for h in range(2):
                                hsl = slice(h * 64, (h + 1) * 64)
                                rs = small.tile([1, 512], f32, tag="rs")
                                nc.vector.reciprocal(rs, psv[h][64:65, :])
                                rb = small.tile([64, 512], f32, tag="rb")
                                nc.gpsimd.partition_broadcast(rb, rs)
                                if h == 0:
                                    # partitions already aligned with gate
                                    tmp = small.tile([64, 512], f32,
                                                     tag="tmp")
                                    nc.vector.tensor_mul(tmp, psv[0][0:64, :],
                                                         rb)
                                    nc.vector.tensor_mul(
                                        gatedT_s[hsl, nsl], tmp,
                                        gT_s[hsl, nsl])
                                else:
                                    tmp = small.tile([64, 512], f32,
                                                     tag="tmp")
                                    nc.vector.tensor_mul(tmp, psv[1][0:64, :],
                                                         rb)
                                    pvn = small.tile([128, 512], f32,
                                                     tag="pvn")
                                    nc.sync.dma_start(out=pvn[hsl, :],
                                                      in_=tmp)
                                    nc.vector.tensor_mul(
                                        gatedT_s[hsl, nsl], pvn[hsl, :],
                                        gT_s[hsl, nsl])
                        # projection for this (b, nh): reuses pv banks
                        for tb in range(b * 16 + nh * 8, b * 16 + nh * 8 + 8):
                            for j in range(2):
                                py_ = psum.tile([128, 512], f32,
                                                tag=f"pv{j}", name="proj")
                                nc.tensor.matmul(
                                    py_,
                                    gatedT_s[:, tb * 128:(tb + 1) * 128],
                                    wp_sb[:, j * 512:(j + 1) * 512],
                                    start=True, stop=True)
                                yt = y_pool.tile([128, 512], y_dt, tag="yt")
                                nc.vector.tensor_copy(yt, py_)
                                nc.sync.dma_start(
                                    out=y[tb * 128:(tb + 1) * 128,
                                          j * 512:(j + 1) * 512],
                                    in_=yt)

    nc.compile()
    return nc


def _get_nc():
    if "nc" not in _cache:
        _cache["nc"] = _build_program()
    return _cache["nc"]


def _get_exec():
    """Compile once; cache a persistent sharded executable.

    Mirrors concourse.bass2jax.run_bass_via_pjrt's multi-core path, but
    keeps the jitted callable (and device-resident zero output buffers)
    alive so repeat kernel() calls skip XLA/walrus recompilation.  No
    donation: the kernel writes every element of y, so the zero buffers
    are never read and can be reused across calls.
    """
    if "exec" in _cache:
        return _cache["exec"]
    import jax
    from jax.experimental.shard_map import shard_map
    from jax.sharding import Mesh, PartitionSpec
    from concourse import mybir
    from concourse.bass2jax import (_bass_exec_p, install_neuronx_cc_hook,
                                    partition_id_tensor)

    nc = _get_nc()
    install_neuronx_cc_hook()
    partition_name = (nc.partition_id_tensor.name
                      if nc.partition_id_tensor else None)
    in_names, out_names, out_avals = [], [], []
    for alloc in nc.m.functions[0].allocations:
        if not isinstance(alloc, mybir.MemoryLocationSet):
            continue
        name = alloc.memorylocations[0].name
        if alloc.kind == "ExternalInput":
            if name != partition_name:
                in_names.append(name)
        elif alloc.kind == "ExternalOutput":
            out_names.append(name)
            out_avals.append(jax.core.ShapedArray(
                tuple(alloc.tensor_shape), mybir.dt.np(alloc.dtype)))
    n_params, n_outs = len(in_names), len(out_names)
    bind_in_names = tuple(in_names + out_names +
                          ([partition_name] if partition_name else []))

    def _body(*args):
        operands = list(args)
        if partition_name is not None:
            operands.append(partition_id_tensor())
        outs = _bass_exec_p.bind(
            *operands,
            out_avals=tuple(out_avals),
            in_names=bind_in_names,
            out_names=tuple(out_names),
            lowering_input_output_aliases=(),
            sim_require_finite=True,
            sim_require_nnan=True,
            nc=nc,
        )
        return tuple(outs)

    devices = jax.devices()[:NCORES]
    mesh = Mesh(np.asarray(devices), ("core",))
    in_specs = (PartitionSpec("core"),) * (n_params + n_outs)
    out_specs = (PartitionSpec("core"),) * n_outs
    sharded = jax.jit(shard_map(_body, mesh=mesh, in_specs=in_specs,
                                out_specs=out_specs, check_rep=False),
                      keep_unused=True)
    zeros_dev = [
        jax.device_put(
            np.zeros((NCORES * a.shape[0], *a.shape[1:]), a.dtype),
            jax.sharding.NamedSharding(mesh, PartitionSpec("core")))
        for a in out_avals]
    reduce_fn = jax.jit(
        lambda a: a.reshape(NCORES, T, C).astype(np.float32).sum(axis=0))
    ex = {"fn": sharded, "in_names": in_names, "out_names": out_names,
          "out_avals": out_avals, "mesh": mesh, "zeros_dev": zeros_dev,
          "spec": PartitionSpec("core"), "reduce": reduce_fn}
    _cache["exec"] = ex
    return ex


def _run_on_device(in_maps):
    """Run the cached executable; returns per-core output dicts."""
    ex = _get_exec()
    concat_in = [
        np.concatenate([np.asarray(in_maps[c][name])
                        for c in range(NCORES)], axis=0)
        for name in ex["in_names"]]
    out = ex["fn"](*concat_in, *ex["zeros_dev"])
    return [
        {name: np.asarray(out[i]).reshape(NCORES, *ex["out_avals"][i].shape)[c]
         for i, name in enumerate(ex["out_names"])}
        for c in range(NCORES)]


def _make_in_maps(x, e, W_qkv, W_s, W_gate, W_proj):
    xT = np.ascontiguousarray(
        x.reshape(T, C).T).astype(BF16)
    eT = np.ascontiguousarray(
        e.reshape(T, C).T).astype(BF16)
    in_maps = []
    for c in range(NCORES):
        fs = slice(F * c, F * (c + 1))
        in_maps.append({
            "xT": xT,
            "eT": eT,
            "wq": np.ascontiguousarray(W_qkv[:, fs]).astype(BF16),
            "wk": np.ascontiguousarray(W_qkv[:, C:][:, fs]).astype(BF16),
            "wv": np.ascontiguousarray(W_qkv[:, 2 * C:][:, fs]).astype(BF16),
            "ws": np.ascontiguousarray(W_s[:, fs]).astype(BF16),
            "wg": np.ascontiguousarray(W_gate[:, fs]).astype(BF16),
            "wp": np.ascontiguousarray(W_proj[fs, :]).astype(BF16),
        })
    return in_maps


def kernel(x, e, W_qkv, W_s, W_gate, W_proj, b_proj):
    ex = _get_exec()
    in_maps = _make_in_maps(np.asarray(x), np.asarray(e), np.asarray(W_qkv),
                            np.asarray(W_s), np.asarray(W_gate),
                            np.asarray(W_proj))
    concat_in = [
        np.concatenate([np.asarray(in_maps[c][name])
                        for c in range(NCORES)], axis=0)
        for name in ex["in_names"]]
    out = ex["fn"](*concat_in, *ex["zeros_dev"])
    iy = ex["out_names"].index("y")
    y_sum = np.asarray(ex["reduce"](out[iy]))   # cross-core partial sum
    y_sum = y_sum + np.asarray(b_proj, dtype=np.float32)
    return y_sum.reshape(B, N, C).astype(np.float32)
